# revision 1
# baseline (speedup 1.0000x reference)
"""Trainium2 kernel: depthwise (channel-multiplier-2) 3x3 conv + wing-swap + add.

Reference computes, for input x (B=32, C=256, H=W=56) and weights w (512,1,3,3):
    y[:, 2i], y[:, 2i+1] = conv3x3(x[:, i], w[2i]), conv3x3(x[:, i], w[2i+1])
    out[:, c] = y[:, 2c] + y[:, 2*swap(c)+1]
where swap() exchanges the two 4-channel wings inside each 8-channel butterfly.
Equivalently:  out[:, c] = conv3x3(x[:, c], w[2c]) + conv3x3(x[:, sc], w[2sc+1]),
sc = swap(c).

Strategy (8 NeuronCores, data-parallel over batch, 4 images/core):
  - channels on SBUF partitions, spatial pixels on the free dim
  - host pre-pads W by 1 col each side (zeros) so every tap reads in-bounds
    and every matmul writes a full, contiguous PSUM window
  - per (image, 128-channel half): 9 per-tap 128x128 block-diagonal matmuls
    (fp16 operands, 1 cycle/column, fp32 PSUM accumulate); the wing swap is
    folded into the per-tap weight matrices host-side (2 nonzeros per
    output-channel column)
  - input rows stream in 4 halo'd chunk tiles per unit for fast start and
    fine-grained DMA/compute overlap; ScalarE evacuates PSUM->SBUF
  - measured ~106 us/core on HW (PE roofline ~93 us; DMA ~54 us),
    absmax rel err ~5e-4 vs the fp32 reference
"""

import sys
from contextlib import ExitStack

import numpy as np

for _p in ("/opt/trn_rl_repo",):
    if _p not in sys.path:
        sys.path.insert(0, _p)

import concourse.bass as bass
import concourse.tile as tile
from concourse import bacc, mybir
from concourse.bass_utils import run_bass_kernel_spmd

B, C, H, W = 32, 256, 56, 56
WP = W + 2  # host-padded row width
N_CORES = 8
B_PER = B // N_CORES  # images per core
P = 128               # partitions = channels per half
HALVES = C // P       # 2
RB = 8                # output rows per PSUM block
NRB = H // RB         # 7
NTAPS = 9
BFLY = 8
WING = BFLY // 2

# center tap first: it always writes the full block, so it carries start=True
TAPS = [(0, 0)] + [
    (dh, dw) for dh in (-1, 0, 1) for dw in (-1, 0, 1) if (dh, dw) != (0, 0)
]

_prog_cache = {}


def _swap_local(m: np.ndarray) -> np.ndarray:
    b, r = m // BFLY, m % BFLY
    wng, pos = r // WING, r % WING
    return b * BFLY + (1 - wng) * WING + pos


def _build_weights(w: np.ndarray) -> np.ndarray:
    """Per-tap block-diagonal stationary matrices.

    Returns (P, HALVES*NTAPS*P) f32; wts[k, (h*9+t)*128 + m] is the weight
    from input channel k (partition) to output channel m for tap t of half h.
    """
    w2 = w.reshape(2 * C, NTAPS).astype(np.float32)
    wts = np.zeros((P, HALVES, NTAPS, P), np.float32)
    m = np.arange(P)
    sl = _swap_local(m)
    for h in range(HALVES):
        cg = h * P + m
        sg = h * P + sl
        wts[m, h, :, m] = w2[2 * cg]          # x[c] * w[2c]
        wts[sl, h, :, m] = w2[2 * sg + 1]     # x[sc] * w[2sc+1]
    return np.ascontiguousarray(wts.reshape(P, HALVES * NTAPS * P))


def _build_program(
    loop_iters: int = 1, timing_mode: bool = False, in_dtype: str = "fp16"
) -> bass.Bass:
    # Bacc (not plain Bass): its compile() runs generate_event_semaphores,
    # which splits multi-wait instructions to satisfy the TRN2 1-wait limit
    nc = bacc.Bacc("TRN2", target_bir_lowering=False, debug=False)
    f32 = mybir.dt.float32
    # input dtype trade-off (all run the PE at 1 cycle/column):
    #   f32r: fp32 bits, rel err ~2.2e-4, but 4-byte DMA + slow weight loads
    #   fp16: rel err ~4.1e-4 (11-bit mantissa; |x|<6, |w|<0.5 -> no overflow),
    #         halves input DMA and enables fast (FWL) weight loads
    #   bf16: rel err ~3.6e-3 (8-bit mantissa), same speed as fp16
    in_dt = {
        "f32r": mybir.dt.float32r,
        "fp16": mybir.dt.float16,
        "bf16": mybir.dt.bfloat16,
    }[in_dtype]
    if timing_mode:
        # benchmark-only build: big tensors stay in device DRAM (garbage
        # contents) so wall-time isn't dominated by axon transfers
        x_d = nc.dram_tensor("x_int", [B_PER, C, H, WP], in_dt).ap()
        o_d = nc.dram_tensor("o_int", [B_PER, C, H * W], f32).ap()
        nc.dram_tensor("tiny", [1, 4], f32, kind="ExternalOutput")
    else:
        x_d = nc.dram_tensor("x", [B_PER, C, H, WP], in_dt, kind="ExternalInput").ap()
        o_d = nc.dram_tensor("out", [B_PER, C, H * W], f32, kind="ExternalOutput").ap()
    w_d = nc.dram_tensor("wts", [P, HALVES * NTAPS, P], in_dt, kind="ExternalInput").ap()

    with tile.TileContext(nc) as tc, ExitStack() as ctx:
        wpool = ctx.enter_context(tc.tile_pool(name="wpool", bufs=1))
        xpool = ctx.enter_context(tc.tile_pool(name="xpool", bufs=8))
        opool = ctx.enter_context(tc.tile_pool(name="opool", bufs=4))
        ppool = ctx.enter_context(tc.tile_pool(name="ppool", bufs=4, space="PSUM"))

        # input rows arrive in 4 halo'd chunk tiles per (image, half) — two
        # row-blocks each (+1 row halo both sides) — so the first matmuls
        # start after ~0.5 MB instead of the full 1.7 MB, and unit
        # boundaries pipeline at chunk granularity. Tile tracks deps at
        # tile granularity, hence separate tiles rather than one split DMA.
        CHUNK_LO = [0, 15, 31, 47]          # first input row of each chunk
        CHUNK_HI = [17, 33, 49, 56]         # one past last input row
        CHUNK_ROWS = 18                     # max rows in any chunk
        wts_sb = {}
        for h in range(HALVES):
            wts_sb[h] = wpool.tile(
                [P, NTAPS, P], in_dt, name=f"wt{h}", tag=f"wt{h}"
            )
        # half-0 weights first: first matmul needs only them + chunk 0
        nc.sync.dma_start(out=wts_sb[0], in_=w_d[:, 0:NTAPS, :])
        if loop_iters > 1:
            nc.sync.dma_start(out=wts_sb[1], in_=w_d[:, NTAPS : 2 * NTAPS, :])

        def body():
            for u in range(B_PER * HALVES):
                img, h = divmod(u, HALVES)
                wt = wts_sb[h]
                chunks = []
                for c in range(4):
                    lo, hi = CHUNK_LO[c], CHUNK_HI[c]
                    xt = xpool.tile([P, CHUNK_ROWS, WP], in_dt,
                                    name=f"xt{u}_{c}", tag="xt")
                    nc.sync.dma_start(
                        out=xt[:, 0 : hi - lo, :],
                        in_=x_d[img, h * P : (h + 1) * P, lo:hi, :],
                    )
                    chunks.append(xt)
                if u == 0 and loop_iters == 1:
                    # half-1 weights queued behind unit 0's input chunks
                    nc.sync.dma_start(out=wts_sb[1], in_=w_d[:, NTAPS : 2 * NTAPS, :])
                for rb in range(NRB):
                    r0 = rb * RB
                    ck = min(rb // 2, 3)
                    lo = CHUNK_LO[ck]
                    xt = chunks[ck]
                    ps = ppool.tile([P, RB, W], f32)
                    for i, (dh, dw) in enumerate(TAPS):
                        rs = max(r0, -dh)
                        re = min(r0 + RB, H - dh)
                        t = (dh + 1) * 3 + (dw + 1)
                        nc.tensor.matmul(
                            ps[:, rs - r0 : re - r0, :],
                            wt[:, t, :],
                            xt[:, rs + dh - lo : re + dh - lo, dw + 1 : dw + 1 + W],
                            start=(i == 0),
                            stop=(i == NTAPS - 1),
                        )
                    ot = opool.tile([P, RB * W], f32)
                    nc.scalar.copy(ot, ps.rearrange("p r c -> p (r c)"))
                    nc.sync.dma_start(
                        out=o_d[img, h * P : (h + 1) * P, r0 * W : r0 * W + RB * W],
                        in_=ot,
                    )

        if loop_iters > 1:
            with tc.For_i(0, loop_iters):
                body()
        else:
            body()
    nc.compile()
    return nc


# on-device conv input dtype: "fp16" measured ~14% faster than "f32r" with
# near-identical accuracy (see _build_program comment)
IN_DTYPE = "fp16"

_NP_DT = {"f32r": np.float32, "fp16": np.float16}


def _np_in_dtype():
    if IN_DTYPE == "bf16":
        import ml_dtypes

        return ml_dtypes.bfloat16
    return _NP_DT[IN_DTYPE]


def _get_program() -> bass.Bass:
    key = f"nc_{IN_DTYPE}"
    if key not in _prog_cache:
        _prog_cache[key] = _build_program(in_dtype=IN_DTYPE)
    return _prog_cache[key]


def _run(x: np.ndarray, w: np.ndarray, **run_kwargs):
    """Shard, run on 8 cores, gather. Returns (output, BassKernelResults)."""
    x = np.asarray(x, np.float32).reshape(B, C, H, W)
    xpad = np.zeros((B, C, H, WP), np.float32)
    xpad[:, :, :, 1 : 1 + W] = x
    wts = _build_weights(np.asarray(w, np.float32))
    if IN_DTYPE != "f32r":
        xpad = xpad.astype(_np_in_dtype())
        wts = wts.astype(_np_in_dtype())

    in_maps = [
        {"x": xpad[c * B_PER : (c + 1) * B_PER], "wts": wts.reshape(P, HALVES * NTAPS, P)}
        for c in range(N_CORES)
    ]
    nc = _get_program()
    res = run_bass_kernel_spmd(nc, in_maps, core_ids=list(range(N_CORES)), **run_kwargs)
    out = np.concatenate([res.results[c]["out"] for c in range(N_CORES)], axis=0)
    return out.reshape(B, C, H, W), res


def kernel(x: np.ndarray, w: np.ndarray) -> np.ndarray:
    out, _ = _run(x, w)
    return out

